# revision 1
# baseline (speedup 1.0000x reference)
"""Grouped GEMM (MoE routing) kernel for Trainium2, 8 NeuronCores.

Problem: Y[o_e:o_e+s_e] = X[o_e:o_e+s_e] @ W[e].T per expert e, with
X [16384, 2048] fp32, W [8, 4096, 2048] fp32, host-static m_sizes/m_offsets.

Sharding: 8-way tensor parallel over OUT_FEATURES (4096 -> 512 per core).
Every core runs the IDENTICAL program over all tokens (the per-expert
segmentation is host-read, compile-time static and the same on all cores);
only the weight slice differs per core. No collectives needed; host
concatenates the per-core [16384, 512] outputs along the feature axis.

Matmul formulation (per 128-token tile, N=512 features, K=2048 contracted
in 16 chunks of 128): out[tok, feat] += XT_chunk[k,tok].T @ WT_chunk[k,feat]
accumulated in one PSUM bank. X is pre-transposed on host to [2048, 16384];
weights pre-transposed/sliced per core to [n_segs, 2048, 512].

Data flows through the PE in float16 by default: 1 cycle/row (same speed
as bf16, 4x faster than fp32), half the DMA bytes of fp32, and ~2.9e-4
rel L2 error on this problem (vs 2.3e-3 bf16, 1.5e-4 float32r -- all
measured on HW). Inputs are N(0,1) so fp16 range is safe. Accumulation
is always fp32 in PSUM. Measured exec: ~455 us typical / 436 us best on
8 cores (96-100% of the ~440 us PE streaming roofline for 2.75e11 FLOP
at 1 cycle/row; residual run-to-run spread tracks chip power state, not
kernel structure).
"""

import os
import time

os.environ.setdefault("NEURON_RT_RESET_CORES", "1")

import numpy as np

import concourse.bass as bass
import concourse.mybir as mybir
import concourse.tile as tile
from concourse import bacc
from concourse import bass_utils

N_CORES = 8
IN_FEATURES = 2048
OUT_FEATURES = 4096
FEAT_PER_CORE = OUT_FEATURES // N_CORES  # 512
KC = IN_FEATURES // 128                  # 16 contraction chunks

_DT = {
    "fp32r": mybir.dt.float32r,
    "bf16": mybir.dt.bfloat16,
    "fp16": mybir.dt.float16,
    "fp32": mybir.dt.float32,
}

# tokens staged in SBUF per X load; 2-byte dtypes get 2 KiB DMA lines at 1024
_TOK_BLOCK = {"fp32r": 512, "fp32": 512, "bf16": 1024, "fp16": 1024}


def _np_dt(tag):
    return mybir.dt.np(_DT[tag])


def build_program(segs, total_tokens, dtype_tag="fp32r", repeat=1,
                  tok_block=None, x_bufs=2, w_bufs=2, o_bufs=4, ps_bufs=8,
                  ramp=()):
    """segs: list of (expert, x_off, y_pos, size). Same program for all cores.

    `ramp`: block sizes for the start of the FIRST segment (e.g. (128, 384))
    so the first matmul starts after a small X load instead of a full
    TOK_BLOCK one -- shaves pipeline-fill latency off a single-shot run."""
    dt = _DT[dtype_tag]
    f32 = mybir.dt.float32
    n_segs = len(segs)
    TOK_BLOCK = tok_block if tok_block is not None else _TOK_BLOCK[dtype_tag]

    def block_sizes(size, first_seg):
        out = []
        done = 0
        if first_seg:
            for r in ramp:
                take = min(r, size - done)
                if take > 0:
                    out.append(take)
                    done += take
        while done < size:
            take = min(TOK_BLOCK, size - done)
            out.append(take)
            done += take
        return out

    nc = bacc.Bacc("TRN2", target_bir_lowering=False, debug=False,
                   num_devices=N_CORES)
    xt = nc.dram_tensor("xt", [IN_FEATURES, total_tokens], dt,
                        kind="ExternalInput").ap()
    wt = nc.dram_tensor("wt", [n_segs, IN_FEATURES, FEAT_PER_CORE], dt,
                        kind="ExternalInput").ap()
    y = nc.dram_tensor("y", [total_tokens, FEAT_PER_CORE], f32,
                       kind="ExternalOutput").ap()

    with tile.TileContext(nc) as tc:
        with (
            tc.tile_pool(name="wp", bufs=w_bufs) as wpool,
            tc.tile_pool(name="xp", bufs=x_bufs) as xpool,
            tc.tile_pool(name="op", bufs=o_bufs) as opool,
            tc.tile_pool(name="pp", bufs=ps_bufs, space="PSUM") as pspool,
        ):
            for _ in range(repeat):
                for s, (e, off, pos, size) in enumerate(segs):
                    w_sb = wpool.tile([128, KC * FEAT_PER_CORE], dt, tag="w")
                    for k in range(KC):
                        nc.sync.dma_start(
                            w_sb[:, k * FEAT_PER_CORE:(k + 1) * FEAT_PER_CORE],
                            wt[s, k * 128:(k + 1) * 128, :],
                        )
                    b0 = 0
                    for blk in block_sizes(size, s == 0):
                        x_sb = xpool.tile([128, KC * TOK_BLOCK], dt, tag="x")
                        for k in range(KC):
                            nc.sync.dma_start(
                                x_sb[:, k * TOK_BLOCK:k * TOK_BLOCK + blk],
                                xt[k * 128:(k + 1) * 128, off + b0:off + b0 + blk],
                            )
                        for t0 in range(0, blk, 128):
                            tt = min(128, blk - t0)
                            ps = pspool.tile([128, FEAT_PER_CORE], f32, tag="ps")
                            for k in range(KC):
                                nc.tensor.matmul(
                                    ps[:tt, :],
                                    x_sb[:, k * TOK_BLOCK + t0:k * TOK_BLOCK + t0 + tt],
                                    w_sb[:, k * FEAT_PER_CORE:(k + 1) * FEAT_PER_CORE],
                                    start=(k == 0),
                                    stop=(k == KC - 1),
                                )
                            o_sb = opool.tile([128, FEAT_PER_CORE], f32, tag="o")
                            nc.vector.tensor_copy(o_sb[:tt, :], ps[:tt, :])
                            nc.sync.dma_start(
                                y[pos + b0 + t0:pos + b0 + t0 + tt, :],
                                o_sb[:tt, :],
                            )
                        b0 += blk

    nc.compile()
    return nc


def make_segments(m_sizes, m_offsets, total_tokens=None):
    """(expert, x_offset, y_concat_position, size) per non-empty expert.

    Mirrors the reference's `input_tokens[o:o+s]` numpy slice semantics:
    the slice length (and hence the concat position advance) is clamped
    to the tokens actually available."""
    sizes = np.asarray(m_sizes).astype(np.int64)
    offsets = np.asarray(m_offsets).astype(np.int64)
    segs = []
    pos = 0
    for e in range(len(sizes)):
        s = int(sizes[e])
        o = int(offsets[e])
        if total_tokens is not None:
            o = min(max(o, 0), total_tokens)
            s = max(0, min(s, total_tokens - o))
        if s > 0:
            segs.append((e, o, pos, s))
        pos += s
    return segs, pos


def make_in_maps(input_tokens, weight_stack, segs, dtype_tag="fp32r"):
    np_dt = _np_dt(dtype_tag)
    X = np.asarray(input_tokens, dtype=np.float32)
    W = np.asarray(weight_stack, dtype=np.float32)
    # cast first (cheaper for 2-byte dtypes), then transpose-copy
    Xc = X.astype(np_dt, copy=False)
    Wc = W.astype(np_dt, copy=False)
    XT = np.ascontiguousarray(Xc.T)  # [2048, T]
    in_maps = []
    for c in range(N_CORES):
        # W[e] is [4096, 2048]; core c needs rows c*512..(c+1)*512 transposed
        # -> [2048, 512] per segment.
        wt_c = np.empty((len(segs), IN_FEATURES, FEAT_PER_CORE), dtype=np_dt)
        for s, (e, _, _, _) in enumerate(segs):
            wt_c[s] = Wc[e, c * FEAT_PER_CORE:(c + 1) * FEAT_PER_CORE, :].T
        in_maps.append({"xt": XT, "wt": wt_c})
    return in_maps


def gather_output(results, total_rows):
    Y = np.empty((total_rows, OUT_FEATURES), dtype=np.float32)
    for c in range(N_CORES):
        Y[:, c * FEAT_PER_CORE:(c + 1) * FEAT_PER_CORE] = \
            results[c]["y"][:total_rows]
    return Y


_PROGRAM_CACHE = {}


def kernel(input_tokens, weight_stack, m_sizes, m_offsets, dtype_tag="fp16"):
    X_shape = tuple(np.asarray(input_tokens).shape)
    W_shape = tuple(np.asarray(weight_stack).shape)
    assert X_shape[1] == IN_FEATURES, X_shape
    assert W_shape[1:] == (OUT_FEATURES, IN_FEATURES), W_shape
    total_tokens = int(X_shape[0])
    segs, total_rows = make_segments(m_sizes, m_offsets, total_tokens)
    if not segs:
        return np.zeros((max(total_rows, 0), OUT_FEATURES), dtype=np.float32)
    key = (tuple(segs), total_tokens, dtype_tag)
    nc = _PROGRAM_CACHE.get(key)
    if nc is None:
        nc = build_program(segs, total_tokens, dtype_tag=dtype_tag)
        _PROGRAM_CACHE[key] = nc
    in_maps = make_in_maps(input_tokens, weight_stack, segs, dtype_tag=dtype_tag)
    # Transient wedged-device INTERNAL errors recover after ~1-2 min on this
    # axon tunnel; retry rather than fail the whole call.
    last_exc = None
    for attempt in range(3):
        if attempt:
            time.sleep(90)
        try:
            res = bass_utils.run_bass_kernel_spmd(
                nc, in_maps, core_ids=list(range(N_CORES)))
            break
        except Exception as e:  # noqa: BLE001 - device wedge is opaque here
            last_exc = e
    else:
        raise last_exc
    return gather_output(res.results, total_rows)

